# revision 2
# baseline (speedup 1.0000x reference)
"""Trainium2 Bass kernel v2 for nn_BiMambaLayer (bidirectional Mamba + gating).

Sharding: channel-split tensor-parallel. Core c = 4b + g handles batch b and
channel group g (512 of 2048 d_inner channels, both directions) over the FULL
T=2048 sequence — the selective scan is exact (no warm-up approximation).

Cross-core collectives per 4-core batch group:
  AllGather    x_gated   (each core computes the gate for its T-quarter)
  AllReduce    xdb = xc @ W_x   (contraction over all d_inner channels)
  ReduceScatter W_out partials  (each core ends with its T-quarter of fwd/bwd)

Channel data lives in [128, 4*2051] "supertiles": 4 channel-tile segments side
by side along the free dim, each segment = 3 zero pads (conv halo + scan
isolation) + 2048 tokens.  dt is memset to 38 at pads so exp(-n*dt) == 0 kills
cross-segment scan leakage; xc pads are 0 so B*dt*xc terms vanish.

Instruction count is the optimization target (the axon dispatch path costs
~20us per NEFF instruction per call, dwarfing on-chip execution).
"""
import os
import sys

for _p in ("/opt/trn_rl_repo", "/root/.axon_site/_ro/trn_rl_repo"):
    if os.path.isdir(_p) and _p not in sys.path:
        sys.path.insert(0, _p)

import numpy as np

D = 1024
DI = 2048
DS = 16
DTR = 64
DCONV = 4
B_SZ = 2
T = 2048
N_CORES = 8
G = 4                # channel groups / cores per batch
CH = DI // G         # 512 channels per core per direction
NSEG = CH // 128     # 4 local channel tiles
Q = T // G           # 512-token quarter
SEG = 3 + T          # 2051
WSUP = NSEG * SEG    # 8204
KD = D // 128        # 8 d_model tiles
GROUPS = [[0, 1, 2, 3], [4, 5, 6, 7]]


def _bf16_np():
    import ml_dtypes
    return np.dtype(ml_dtypes.bfloat16)


def build_nc(generic_A=False):
    import concourse.bass as bass
    import concourse.bacc as bacc
    import concourse.mybir as mybir
    import concourse.tile as tile

    BF = mybir.dt.bfloat16
    F32 = mybir.dt.float32
    AF = mybir.ActivationFunctionType
    OP = mybir.AluOpType

    import concourse.tile_sem_assignment as _tsa
    _tsa.NUM_SWDGE_GLOBAL_SEMS = 1

    nc = bacc.Bacc(trn_type="TRN2")
    AP = bass.AP

    # ---- I/O ----
    dram = {}
    dram["xq"] = nc.dram_tensor("xq", [128, KD * Q], BF, kind="ExternalInput")
    dram["eu"] = nc.dram_tensor("eu", [1, Q], F32, kind="ExternalInput")
    dram["bdelta"] = nc.dram_tensor("bdelta", [128, KD], F32, kind="ExternalInput")
    dram["wdelta"] = nc.dram_tensor("wdelta", [128, KD * KD * 128], BF, kind="ExternalInput")
    dram["wpf"] = nc.dram_tensor("wpf", [128, KD * KD * 128], BF, kind="ExternalInput")
    dram["wpb"] = nc.dram_tensor("wpb", [128, KD * KD * 128], BF, kind="ExternalInput")
    dram["bproj"] = nc.dram_tensor("bproj", [128, KD], F32, kind="ExternalInput")
    for d in ("f", "b"):
        dram[f"win_{d}"] = nc.dram_tensor(f"win_{d}", [128, KD * 8 * 128], BF, kind="ExternalInput")
        dram[f"wx_{d}"] = nc.dram_tensor(f"wx_{d}", [128, NSEG * 96], BF, kind="ExternalInput")
        dram[f"wdt_{d}"] = nc.dram_tensor(f"wdt_{d}", [64, CH], BF, kind="ExternalInput")
        dram[f"bdt_{d}"] = nc.dram_tensor(f"bdt_{d}", [128, NSEG], F32, kind="ExternalInput")
        dram[f"smallw_{d}"] = nc.dram_tensor(f"smallw_{d}", [128, NSEG * 4], F32, kind="ExternalInput")
        dram[f"smallf_{d}"] = nc.dram_tensor(f"smallf_{d}", [128, NSEG * 2], F32, kind="ExternalInput")
        dram[f"wout_{d}"] = nc.dram_tensor(f"wout_{d}", [128, NSEG * KD * 128], BF, kind="ExternalInput")
        if generic_A:
            dram[f"negA_{d}"] = nc.dram_tensor(f"negA_{d}", [128, NSEG * DS], BF, kind="ExternalInput")
    o_all = nc.dram_tensor("oall", [3 * D, Q], F32, kind="ExternalOutput")
    o_out = o_all[0:D, :]
    o_fwd = o_all[D:2 * D, :]
    o_bwd = o_all[2 * D:3 * D, :]

    def dap(handle, offset, dims):
        a = handle[:]
        return AP(tensor=a.tensor, offset=a.offset + offset, ap=[list(x) for x in dims])

    def tap(tile_ap, offset, dims):
        return AP(tensor=tile_ap.tensor, offset=tile_ap.offset + offset,
                  ap=[list(x) for x in dims])

    def rev_view(ap, n):
        return AP(tensor=ap.tensor, offset=ap.offset + (n - 1) * ap.ap[-1][0],
                  ap=[list(ap.ap[0]), [-ap.ap[-1][0], n]])

    with tile.TileContext(nc) as tc:
        with (
            tc.tile_pool(name="psum", bufs=8, space="PSUM") as PS,
            tc.tile_pool(name="pp", bufs=1) as P,
            tc.tile_pool(name="sc", bufs=1) as S,
            tc.tile_pool(name="sc2", bufs=2) as S2,
            tc.tile_pool(name="st", bufs=1) as ST,
            tc.tile_pool(name="st2", bufs=2) as ST2,
            tc.tile_pool(name="dram", bufs=1, space="DRAM") as DP,
        ):
            # DRAM bounce buffers for collectives
            ag_in = DP.tile([D, Q], BF, name="ag_in", tag="ag_in")
            ag_out = DP.tile([G * D, Q], BF, name="ag_out", tag="ag_out")
            cdram = {}
            for d in ("f", "b"):
                cdram[f"xdb_in_{d}"] = DP.tile([96, T], F32, name=f"xdbi{d}", tag=f"xdbi{d}")
                cdram[f"xdb_out_{d}"] = DP.tile([96, T], F32, name=f"xdbo{d}", tag=f"xdbo{d}")
                cdram[f"bc_{d}"] = DP.tile([32, T], BF, name=f"bc{d}", tag=f"bc{d}")
                cdram[f"rs_in_{d}"] = DP.tile([G * D, Q], F32, name=f"rsi{d}", tag=f"rsi{d}")
                cdram[f"rs_out_{d}"] = DP.tile([D, Q], F32, name=f"rso{d}", tag=f"rso{d}")

            # ---- small persistent params ----
            bdelta_t = P.tile([128, KD], F32, name="bdelta", tag="bdelta")
            nc.sync.dma_start(out=bdelta_t, in_=dram["bdelta"][:, :])
            bproj_t = P.tile([128, KD], F32, name="bproj", tag="bproj")
            nc.sync.dma_start(out=bproj_t, in_=dram["bproj"][:, :])
            prm = {}
            for d in ("f", "b"):
                for nm, w in (("bdt", NSEG), ("smallw", NSEG * 4), ("smallf", NSEG * 2)):
                    t = P.tile([128, w], F32, name=f"{nm}_{d}", tag=f"{nm}_{d}")
                    nc.sync.dma_start(out=t, in_=dram[f"{nm}_{d}"][:, :])
                    prm[f"{nm}_{d}"] = t
                t = P.tile([64, CH], BF, name=f"wdt_{d}", tag="wdt")
                nc.sync.dma_start(out=t, in_=dram[f"wdt_{d}"][:, :])
                prm[f"wdt_{d}"] = t
                t = P.tile([128, NSEG * 96], BF, name=f"wx_{d}", tag="wx")
                nc.sync.dma_start(out=t, in_=dram[f"wx_{d}"][:, :])
                prm[f"wx_{d}"] = t

            # ================= gate (T-quarter) =================
            eu_t = S2.tile([128, Q], F32, name="eu", tag="xc_da")
            nc.sync.dma_start(out=eu_t, in_=dap(dram["eu"], 0, [[0, 128], [1, Q]]))
            xq_t = P.tile([128, KD * Q], BF, name="xq", tag="xq_fo")
            nc.sync.dma_start(out=xq_t, in_=dram["xq"][:, :])
            wdl = P.tile([128, KD * KD * 128], BF, name="wdl", tag="bigw1")
            nc.sync.dma_start(out=wdl, in_=dram["wdelta"][:, :])
            xgq = P.tile([128, KD * Q], BF, name="xgq", tag="xgq_bo")

            pss = [PS.tile([128, Q], F32, name="gps", tag="mm") for _ in range(KD)]
            for k in range(KD):
                for m in range(KD):
                    nc.tensor.matmul(pss[m], wdl[:, 128 * (KD * k + m):128 * (KD * k + m) + 128],
                                     xq_t[:, Q * k:Q * k + Q],
                                     start=(k == 0), stop=(k == KD - 1))
            # gate = sigmoid(ln(softplus(p)) + eu); softplus(p) = -ln(sigmoid(-p))
            # bdelta holds -b_delta so sigmoid-evac computes sigmoid(-p)
            gstg = ST.tile([128, KD * Q], BF, name="gstg", tag="wstg")
            for m in range(KD):
                nc.scalar.activation(gstg[:, Q * m:Q * m + Q], pss[m], AF.Sigmoid,
                                     bias=bdelta_t[:, m:m + 1], scale=-1.0)
            nc.scalar.activation(gstg, gstg, AF.Ln)
            nc.scalar.activation(gstg, gstg, AF.Ln, scale=-1.0)
            nc.vector.tensor_add(
                tap(gstg[:], 0, [[gstg[:].ap[0][0], 128], [Q, KD], [1, Q]]),
                tap(gstg[:], 0, [[gstg[:].ap[0][0], 128], [Q, KD], [1, Q]]),
                tap(eu_t[:], 0, [[eu_t[:].ap[0][0], 128], [0, KD], [1, Q]]))
            nc.scalar.activation(gstg, gstg, AF.Sigmoid)
            nc.vector.tensor_mul(xgq, xq_t, gstg)
            # stage quarter -> ag_in [D, Q]
            nc.sync.dma_start(
                out=dap(ag_in, 0, [[Q, 128], [128 * Q, KD], [1, Q]]),
                in_=tap(xgq[:], 0, [[xgq[:].ap[0][0], 128], [Q, KD], [1, Q]]))
            if "ag" in cheat:
                nc.sync.dma_start(out=ag_out[0:D, :], in_=ag_in[:, :])
                nc.sync.dma_start(out=ag_out[D:2 * D, :], in_=ag_in[:, :])
                nc.sync.dma_start(out=ag_out[2 * D:3 * D, :], in_=ag_in[:, :])
                nc.sync.dma_start(out=ag_out[3 * D:4 * D, :], in_=ag_in[:, :])
            else:
                nc.gpsimd.collective_compute(
                    "AllGather", OP.bypass, ins=[ag_in[:, :]], outs=[ag_out[:, :]],
                    replica_groups=GROUPS)

            # ================= per-direction pipeline =================
            fo_tiles = {}
            for d in ("f", "b"):
                win_t = P.tile([128, KD * 8 * 128], BF, name=f"win{d}", tag="bigw1")
                nc.sync.dma_start(out=win_t, in_=dram[f"win_{d}"][:, :])

                xi = S.tile([128, WSUP], BF, name=f"xi{d}", tag="xi")
                nc.vector.memset(xi, 0.0)
                zt = P.tile([128, WSUP], BF, name=f"zt{d}", tag="zt")
                nc.vector.memset(zt, 0.0)

                # ---- W_in over 4 chunks; evac xi (m<4) and silu(z) (m>=4) ----
                for c in range(G):
                    xgc = ST.tile([128, KD * Q], BF, name="xgc", tag="xgc")
                    if d == "f":
                        nc.sync.dma_start(
                            out=tap(xgc[:], 0, [[xgc[:].ap[0][0], 128], [Q, KD], [1, Q]]),
                            in_=dap(ag_out, (D * c) * Q,
                                    [[Q, 128], [128 * Q, KD], [1, Q]]))
                    else:
                        for k in range(KD):
                            nc.sync.dma_start(
                                out=xgc[:, Q * k:Q * k + Q],
                                in_=dap(ag_out, (D * (G - 1 - c) + 128 * k) * Q + (Q - 1),
                                        [[Q, 128], [-1, Q]]))
                    psw = [PS.tile([128, Q], F32, name="wps", tag="mm") for _ in range(8)]
                    for k in range(KD):
                        for m in range(8):
                            nc.tensor.matmul(
                                psw[m], win_t[:, 128 * (8 * k + m):128 * (8 * k + m) + 128],
                                xgc[:, Q * k:Q * k + Q],
                                start=(k == 0), stop=(k == KD - 1))
                    for m in range(4):
                        nc.scalar.activation(
                            xi[:, SEG * m + 3 + Q * c:SEG * m + 3 + Q * c + Q],
                            psw[m], AF.Copy)
                    for m in range(4):
                        nc.scalar.activation(
                            zt[:, SEG * m + 3 + Q * c:SEG * m + 3 + Q * c + Q],
                            psw[4 + m], AF.Silu)

                # ---- conv + silu -> xc ----
                xc = S2.tile([128, WSUP], BF, name=f"xc{d}", tag="xc_da")
                nc.vector.memset(xc, 0.0)
                acc = S.tile([128, WSUP], BF, name=f"cacc{d}", tag="bbc")
                sw = prm[f"smallw_{d}"]
                sf = prm[f"smallf_{d}"]
                for s in range(NSEG):
                    ov = acc[:, SEG * s + 3:SEG * s + 3 + T]
                    nc.vector.tensor_scalar_mul(ov, xi[:, SEG * s:SEG * s + T],
                                                sw[:, 4 * s:4 * s + 1])
                    for j in range(1, 4):
                        nc.vector.scalar_tensor_tensor(
                            ov, xi[:, SEG * s + j:SEG * s + j + T],
                            sw[:, 4 * s + j:4 * s + j + 1], ov, OP.mult, OP.add)
                    nc.scalar.activation(xc[:, SEG * s + 3:SEG * s + 3 + T], ov,
                                         AF.Silu, bias=sf[:, 2 * s:2 * s + 1])

                # ---- xdb = W_x^T xc (partial) -> AllReduce ----
                xstg = ST.tile([96, T], F32, name="xstg", tag="wstg")
                for c in range(G):
                    ps96 = PS.tile([96, Q], F32, name="xps", tag="mm")
                    for k in range(NSEG):
                        nc.tensor.matmul(ps96, prm[f"wx_{d}"][:, 96 * k:96 * k + 96],
                                         xc[:, SEG * k + 3 + Q * c:SEG * k + 3 + Q * c + Q],
                                         start=(k == 0), stop=(k == NSEG - 1))
                    nc.scalar.activation(xstg[:, Q * c:Q * c + Q], ps96, AF.Copy)
                nc.sync.dma_start(out=cdram[f"xdb_in_{d}"][:, :], in_=xstg)
                if "ar" in cheat:
                    nc.sync.dma_start(out=cdram[f"xdb_out_{d}"][:, :],
                                      in_=cdram[f"xdb_in_{d}"][:, :])
                else:
                    nc.gpsimd.collective_compute(
                        "AllReduce", OP.add, ins=[cdram[f"xdb_in_{d}"][:, :]],
                        outs=[cdram[f"xdb_out_{d}"][:, :]], replica_groups=GROUPS)
                xall = ST.tile([96, T], F32, name="xall", tag="wstg")
                nc.sync.dma_start(out=xall, in_=cdram[f"xdb_out_{d}"][:, :])
                xbf = ST.tile([96, T], BF, name="xbf", tag="dtlo")
                nc.vector.tensor_copy(xbf, xall)
                nc.sync.dma_start(out=cdram[f"bc_{d}"][:, :], in_=xbf[64:96, :])

                # ---- dt supertile (negated): dtw = ln(sigmoid(-(raw+b_dt))) = -dt
                # pads: memset exp(-38) -> Ln gives -38 -> dA = exp((n+1)*dtw) = 0
                dtw = S.tile([128, WSUP], BF, name=f"dtw{d}", tag="dtw_ostg")
                nc.vector.memset(dtw, 3.139e-17)
                for m in range(NSEG):
                    for c in range(G):
                        ps = PS.tile([128, Q], F32, name="dps", tag="mm")
                        nc.tensor.matmul(ps, prm[f"wdt_{d}"][:, 128 * m:128 * m + 128],
                                         xbf[0:64, Q * c:Q * c + Q], start=True, stop=True)
                        nc.scalar.activation(
                            dtw[:, SEG * m + 3 + Q * c:SEG * m + 3 + Q * c + Q],
                            ps, AF.Sigmoid, bias=prm[f"bdt_{d}"][:, m:m + 1], scale=-1.0)
                nc.scalar.activation(dtw, dtw, AF.Ln)

                # dtx = dt * xc = (-1 * dtw) * xc
                dtx = S.tile([128, WSUP], BF, name=f"dtx{d}", tag="dtx")
                nc.vector.scalar_tensor_tensor(dtx, dtw, -1.0, xc, OP.mult, OP.mult)
                y = S.tile([128, WSUP], BF, name=f"y{d}", tag="y")
                nc.vector.memset(y, 0.0)
                for s in range(NSEG):
                    nc.vector.tensor_scalar_mul(
                        y[:, SEG * s + 3:SEG * s + 3 + T],
                        xc[:, SEG * s + 3:SEG * s + 3 + T],
                        sf[:, 2 * s + 1:2 * s + 2])

                # ---- selective scan over states ----
                bbc = S.tile([128, WSUP], BF, name=f"bbc{d}", tag="bbc")
                nc.vector.memset(bbc, 0.0)
                cbc = S.tile([128, WSUP], BF, name=f"cbc{d}", tag="cbc")
                nc.vector.memset(cbc, 0.0)
                for n in range(DS):
                    nc.sync.dma_start(
                        out=tap(bbc[:], 3, [[bbc[:].ap[0][0], 128], [SEG, NSEG], [1, T]]),
                        in_=dap(cdram[f"bc_{d}"], n * T, [[0, 128], [0, NSEG], [1, T]]))
                    nc.sync.dma_start(
                        out=tap(cbc[:], 3, [[cbc[:].ap[0][0], 128], [SEG, NSEG], [1, T]]),
                        in_=dap(cdram[f"bc_{d}"], (DS + n) * T, [[0, 128], [0, NSEG], [1, T]]))
                    dA = S2.tile([128, WSUP], BF, name="dA", tag="xc_da")
                    if generic_A:
                        nc.sync.dma_start(
                            out=tap(dA[:], 0, [[dA[:].ap[0][0], 128], [SEG, NSEG], [1, SEG]]),
                            in_=dap(dram[f"negA_{d}"], n,
                                    [[NSEG * DS, 128], [DS, NSEG], [0, SEG]]))
                        nc.vector.scalar_tensor_tensor(dA, dA, -1.0, dtw, OP.mult, OP.mult)
                        nc.scalar.activation(dA, dA, AF.Exp)
                    else:
                        nc.scalar.activation(dA, dtw, AF.Exp, scale=float(n + 1))
                    nc.vector.tensor_mul(bbc, dtx, bbc)
                    h = S.tile([128, WSUP], BF, name="h", tag="xi")
                    nc.vector.tensor_tensor_scan(h, dA, bbc, 0.0, OP.mult, OP.add)
                    nc.vector.tensor_mul(h, h, cbc)
                    nc.vector.tensor_add(y, y, h)

                # ---- y2 = (y + xc*Dp) * silu(z) ----
                nc.vector.tensor_mul(y, y, zt)

                # ---- W_out partial -> ReduceScatter over T-quarters ----
                wout_t = P.tile([128, NSEG * KD * 128], BF, name=f"wo{d}", tag="woutw")
                nc.sync.dma_start(out=wout_t, in_=dram[f"wout_{d}"][:, :])
                for c in range(G):
                    pso = [PS.tile([128, Q], F32, name="ops", tag="mm") for _ in range(KD)]
                    for k in range(NSEG):
                        for m in range(KD):
                            nc.tensor.matmul(
                                pso[m], wout_t[:, 128 * (KD * k + m):128 * (KD * k + m) + 128],
                                y[:, SEG * k + 3 + Q * c:SEG * k + 3 + Q * c + Q],
                                start=(k == 0), stop=(k == NSEG - 1))
                    cblk = c if d == "f" else (G - 1 - c)
                    for half in range(2):
                        wst = ST.tile([128, 4 * Q], F32, name="wst", tag="wstg")
                        for m2 in range(4):
                            m = half * 4 + m2
                            dst = wst[:, Q * m2:Q * m2 + Q]
                            if d == "b":
                                dst = rev_view(dst, Q)
                            nc.scalar.activation(dst, pso[m], AF.Copy)
                        nc.sync.dma_start(
                            out=dap(cdram[f"rs_in_{d}"],
                                    (D * cblk + 512 * half) * Q,
                                    [[Q, 128], [128 * Q, 4], [1, Q]]),
                            in_=tap(wst[:], 0, [[wst[:].ap[0][0], 128], [Q, 4], [1, Q]]))
                if "rs" in cheat:
                    nc.sync.dma_start(out=cdram[f"rs_out_{d}"][:, :],
                                      in_=cdram[f"rs_in_{d}"][0:D, :])
                else:
                    nc.gpsimd.collective_compute(
                        "ReduceScatter", OP.add, ins=[cdram[f"rs_in_{d}"][:, :]],
                        outs=[cdram[f"rs_out_{d}"][:, :]], replica_groups=GROUPS)
                fo = P.tile([128, KD * Q], BF, name=f"fo{d}",
                            tag=("xq_fo" if d == "f" else "xgq_bo"))
                for half in range(2):
                    fst = ST.tile([128, 4 * Q], F32, name="fst", tag="wstg")
                    nc.sync.dma_start(
                        out=fst, in_=dap(cdram[f"rs_out_{d}"], (512 * half) * Q,
                                         [[Q, 128], [128 * Q, 4], [1, Q]]))
                    nc.vector.tensor_copy(fo[:, 4 * Q * half:4 * Q * half + 4 * Q], fst)
                fo_tiles[d] = fo
                odst = o_fwd if d == "f" else o_bwd
                nc.sync.dma_start(out=odst, in_=cdram[f"rs_out_{d}"][:, :])

            # ================= proj =================
            psp = [PS.tile([128, Q], F32, name="pps", tag="mm") for _ in range(KD)]
            for k in range(2 * KD):
                wpk = ST.tile([128, KD * 128], BF, name="wpk", tag="xgc")
                srcw = dram["wpf"] if k < KD else dram["wpb"]
                kb = k % KD
                nc.sync.dma_start(out=wpk, in_=srcw[:, KD * 128 * kb:KD * 128 * (kb + 1)])
                rhs = fo_tiles["f"] if k < KD else fo_tiles["b"]
                for m in range(KD):
                    nc.tensor.matmul(psp[m], wpk[:, 128 * m:128 * m + 128],
                                     rhs[:, Q * kb:Q * kb + Q],
                                     start=(k == 0), stop=(k == 2 * KD - 1))
            ostg = S.tile([128, KD * Q], F32, name="ostg", tag="dtw_ostg")
            for m in range(KD):
                nc.scalar.activation(ostg[:, Q * m:Q * m + Q], psp[m], AF.Identity,
                                     bias=bproj_t[:, m:m + 1], scale=1.0)
            nc.sync.dma_start(
                out=dap(o_out, 0, [[Q, 128], [128 * Q, KD], [1, Q]]),
                in_=tap(ostg[:], 0, [[ostg[:].ap[0][0], 128], [Q, KD], [1, Q]]))

    if not nc.is_finalized():
        nc.finalize()
    return nc


def prep_inputs(inputs):
    """Host-side packing: full inputs -> per-core in_maps."""
    bf16 = _bf16_np()
    x = np.asarray(inputs["x"], np.float32)
    u = np.asarray(inputs["u"], np.float32)
    alpha = np.float32(inputs["alpha"])

    def lhsT_pack(w, nk, nm):
        # w [nk*128, nm*128] -> [128, nk*nm*128]: col 128*(nm*k+m)+c = w[128k+p, 128m+c]
        return np.ascontiguousarray(
            w.reshape(nk, 128, nm, 128).transpose(1, 0, 2, 3).reshape(128, -1)
        ).astype(bf16)

    wmap = {
        "bdelta": np.ascontiguousarray(
            -np.asarray(inputs["b_delta"], np.float32).reshape(KD, 128).T),
        "wdelta": lhsT_pack(np.asarray(inputs["W_delta"], np.float32), KD, KD),
        "wpf": lhsT_pack(np.asarray(inputs["W_proj"], np.float32)[:D], KD, KD),
        "wpb": lhsT_pack(np.asarray(inputs["W_proj"], np.float32)[D:], KD, KD),
        "bproj": np.ascontiguousarray(
            np.asarray(inputs["b_proj"], np.float32).reshape(KD, 128).T),
    }
    gmaps = [dict(wmap) for _ in range(G)]
    for d, pre in (("f", "fwd_"), ("b", "bwd_")):
        W_in = np.asarray(inputs[pre + "W_in"], np.float32)      # [D, 2*DI]
        conv_w = np.asarray(inputs[pre + "conv_w"], np.float32)  # [DI, 4]
        conv_b = np.asarray(inputs[pre + "conv_b"], np.float32)
        W_x = np.asarray(inputs[pre + "W_x"], np.float32)        # [DI, 96]
        W_dt = np.asarray(inputs[pre + "W_dt"], np.float32)      # [64, DI]
        b_dt = np.asarray(inputs[pre + "b_dt"], np.float32)
        negA = -np.exp(np.asarray(inputs[pre + "A_log"], np.float32))  # [DI, DS]
        Dp = np.asarray(inputs[pre + "Dp"], np.float32)
        W_out = np.asarray(inputs[pre + "W_out"], np.float32)    # [DI, D]
        for g in range(G):
            ch = slice(CH * g, CH * (g + 1))
            m = gmaps[g]
            # win: xi cols then z cols, as 8 m-tiles of 128
            wsl = np.concatenate([W_in[:, ch], W_in[:, DI + CH * g:DI + CH * (g + 1)]], 1)
            m[f"win_{d}"] = lhsT_pack(wsl, KD, 8)
            m[f"wx_{d}"] = np.ascontiguousarray(
                W_x[ch].reshape(NSEG, 128, 96).transpose(1, 0, 2).reshape(128, -1)
            ).astype(bf16)
            m[f"wdt_{d}"] = np.ascontiguousarray(W_dt[:, ch]).astype(bf16)
            m[f"bdt_{d}"] = np.ascontiguousarray(
                -b_dt[ch].reshape(NSEG, 128).T)
            m[f"smallw_{d}"] = np.ascontiguousarray(
                conv_w[ch].reshape(NSEG, 128, 4).transpose(1, 0, 2).reshape(128, -1))
            sfl = np.stack([conv_b[ch], Dp[ch]], -1)             # [CH, 2]
            m[f"smallf_{d}"] = np.ascontiguousarray(
                sfl.reshape(NSEG, 128, 2).transpose(1, 0, 2).reshape(128, -1))
            m[f"negA_{d}"] = np.ascontiguousarray(
                negA[ch].reshape(NSEG, 128, DS).transpose(1, 0, 2).reshape(128, -1)
            ).astype(bf16)
            m[f"wout_{d}"] = lhsT_pack(W_out[ch], NSEG, KD)

    in_maps = []
    for core in range(N_CORES):
        b, g = core // G, core % G
        m = dict(gmaps[g])
        xsl = x[b, Q * g:Q * (g + 1)]                            # [Q, D]
        m["xq"] = np.ascontiguousarray(
            xsl.reshape(Q, KD, 128).transpose(2, 1, 0).reshape(128, -1)).astype(bf16)
        m["eu"] = np.ascontiguousarray(
            (-alpha * u[b, Q * g:Q * (g + 1), 0]).reshape(1, Q))
        in_maps.append(m)
    return in_maps


def uses_fast_A(inputs):
    ar = np.arange(1, DS + 1, dtype=np.float32)
    for pre in ("fwd_", "bwd_"):
        A = np.exp(np.asarray(inputs[pre + "A_log"], np.float32))
        if not np.allclose(A, np.broadcast_to(ar, (DI, DS)), rtol=1e-5, atol=1e-5):
            return False
    return True


def assemble(results):
    out = np.zeros((B_SZ, T, D), np.float32)
    fwd = np.zeros((B_SZ, T, D), np.float32)
    bwd = np.zeros((B_SZ, T, D), np.float32)
    for core in range(N_CORES):
        b, g = core // G, core % G
        oall = np.asarray(results[core]["oall"], np.float32)
        out[b, Q * g:Q * (g + 1)] = oall[0:D].T
        fwd[b, Q * g:Q * (g + 1)] = oall[D:2 * D].T
        bwd[b, Q * g:Q * (g + 1)] = oall[2 * D:3 * D].T
    return out, fwd, bwd


_NC_CACHE = {}


def kernel(**inputs):
    from concourse.bass_utils import run_bass_kernel_spmd

    fast = uses_fast_A(inputs)
    key = "nc_fast" if fast else "nc_gen"
    if key not in _NC_CACHE:
        _NC_CACHE[key] = build_nc(generic_A=not fast)
    nc = _NC_CACHE[key]
    in_maps = prep_inputs(inputs)
    if fast:
        for m in in_maps:
            m.pop("negA_f", None)
            m.pop("negA_b", None)
    res = run_bass_kernel_spmd(nc, in_maps, list(range(N_CORES)))
    return assemble(res.results)
